# revision 1
# baseline (speedup 1.0000x reference)
"""Trainium2 Bass kernel for HeatmapMaxDetBlock (argmax + local refinement).

Computes, for x[B, C, H, W]:
    scores = max over (H*W); idx = argmax; px = idx % W, py = idx // W (masked
    by score > 0); quarter-pixel refinement by sign of neighbor differences.
Returns [B, C, 3] = (px, py, scores).

Strategy (pure data parallel over 8 NeuronCores, batch-sharded):
  phase 1: stream the whole shard through SBUF once; one DVE reduce_max per
           [128, SEGW] tile gives per-(row, segment) maxima.
  phase 2: tiny ops — PE-transpose the maxima, per-row max + winning segment,
           one indirect-DMA window gather per row group, max_index for the
           exact in-segment position, a second indirect gather of the +-W
           neighborhood, then the scalar-ish refinement math.
"""

import sys
from contextlib import ExitStack
from dataclasses import dataclass

import numpy as np

for _p in ("/opt/trn_rl_repo",):
    if _p not in sys.path:
        sys.path.insert(0, _p)

import concourse.bass as bass  # noqa: E402
import concourse.tile as tile  # noqa: E402
from concourse import bacc, mybir  # noqa: E402
from concourse.masks import make_identity  # noqa: E402

F32 = mybir.dt.float32
U32 = mybir.dt.uint32
AX = mybir.AxisListType
OP = mybir.AluOpType


@dataclass(frozen=True)
class Cfg:
    B: int = 64
    C: int = 17
    H: int = 256
    W: int = 192
    ncores: int = 8
    P: int = 128
    NSEG: int = 64
    MD: int = 4  # tile-columns merged per DMA
    FRONT: int = 256
    REAR: int = 512

    @property
    def BP(self):  # batches per core
        return self.B // self.ncores

    @property
    def R(self):  # heatmap rows per core
        return self.BP * self.C

    @property
    def HWm(self):
        return self.H * self.W

    @property
    def SEGW(self):
        return self.HWm // self.NSEG

    @property
    def RPT(self):  # rows per tile
        return self.P // self.NSEG

    @property
    def NT(self):  # tiles per core
        return self.R // self.RPT

    @property
    def MARG(self):
        return self.W + 2

    @property
    def WINW(self):
        return self.SEGW + 2 * self.MARG

    @property
    def NBW(self):
        return 2 * self.W + 1

    @property
    def SHN(self):
        return self.R * self.HWm

    @property
    def NPAD(self):
        return self.FRONT + self.SHN + self.REAR


CFG = Cfg()


def build_program(cfg: Cfg):
    c = cfg
    assert c.P % c.NSEG == 0 and c.R % c.RPT == 0 and c.HWm % c.NSEG == 0
    assert c.R <= c.P or c.R - c.P in range(0, 17), (
        "group B must fit in one 16-partition pad"
    )
    assert c.FRONT >= c.MARG and c.REAR >= c.MARG
    assert 8 <= c.SEGW <= 16384
    assert c.NT % c.MD == 0 and c.NT <= c.P
    GA = min(c.P, c.R)
    assert GA % c.RPT == 0

    nc = bacc.Bacc(
        "TRN2", target_bir_lowering=False, debug=False, num_devices=c.ncores
    )
    xh = nc.dram_tensor("x", [c.NPAD], F32, kind="ExternalInput").ap()
    rbh = nc.dram_tensor("rowbase", [c.NT, c.RPT], F32, kind="ExternalInput").ap()
    irh = nc.dram_tensor("iotarev", [c.NT, c.P], F32, kind="ExternalInput").ap()
    oh = nc.dram_tensor("out", [c.R, 3], F32, kind="ExternalOutput").ap()

    with ExitStack() as ctx:
        tc = ctx.enter_context(tile.TileContext(nc))
        xpool = ctx.enter_context(tc.tile_pool(name="xp", bufs=3))
        sp = ctx.enter_context(tc.tile_pool(name="sp", bufs=1))
        pp = ctx.enter_context(tc.tile_pool(name="pp", bufs=1, space="PSUM"))

        # ---- phase 1: per-(row, segment) maxima ------------------------------
        # DMA tile g: [P, MD*SEGW]; partition p = RPT-row j * NSEG + seg s;
        # free = MD tile-columns (row-groups) of SEGW. One reduce per DMA
        # yields MD columns of M.
        M = sp.tile([c.P, c.P], F32, tag="M")
        nc.vector.memset(M[:], 0.0)
        ndma = c.NT // c.MD
        rows_per_dma = c.MD * c.RPT
        for g in range(ndma):
            xt = xpool.tile([c.P, c.MD * c.SEGW], F32, tag="xt")
            off = c.FRONT + g * rows_per_dma * c.HWm
            src = bass.AP(
                xh.tensor,
                off,
                [
                    [c.HWm, c.RPT],
                    [c.SEGW, c.NSEG],
                    [c.RPT * c.HWm, c.MD],
                    [1, c.SEGW],
                ],
            )
            eng = nc.sync if g % 2 == 0 else nc.scalar
            eng.dma_start(
                out=xt[:].rearrange("p (m u) -> p m u", m=c.MD), in_=src
            )
            nc.vector.reduce_max(
                out=M[:, g * c.MD : (g + 1) * c.MD],
                in_=xt[:].rearrange("p (m u) -> p m u", m=c.MD),
                axis=AX.X,
            )

        # ---- phase 2: find row max + winning segment -------------------------
        ident = sp.tile([c.P, c.P], F32, tag="ident")
        make_identity(nc, ident[:])
        mtp = pp.tile([c.P, c.P], F32)
        nc.tensor.transpose(out=mtp[:], in_=M[:], identity=ident[:])
        MT = sp.tile([c.P, c.P], F32, tag="MT")
        nc.vector.tensor_copy(out=MT[:], in_=mtp[:])
        # MT[t, j*NSEG + s] = max of (row RPT*t + j, segment s)
        MT3 = MT[0 : c.NT].rearrange("p (j s) -> p j s", j=c.RPT)

        scores = sp.tile([c.NT, c.RPT], F32, tag="scores")
        nc.vector.reduce_max(out=scores[:], in_=MT3, axis=AX.X)

        irt = sp.tile([c.NT, c.P], F32, tag="irt")
        nc.sync.dma_start(out=irt[:], in_=irh[:])
        rbt = sp.tile([c.NT, c.RPT], F32, tag="rbt")
        nc.sync.dma_start(out=rbt[:], in_=rbh[:])

        mk16 = sp.tile([c.NT, c.P], F32, tag="mk16")
        mk16_3 = mk16[:].rearrange("p (j s) -> p j s", j=c.RPT)
        nc.vector.tensor_tensor(
            out=mk16_3,
            in0=MT3,
            in1=scores[:, :, None].to_broadcast([c.NT, c.RPT, c.NSEG]),
            op=OP.is_equal,
        )
        nc.vector.tensor_tensor(
            out=mk16_3,
            in0=mk16_3,
            in1=irt[:].rearrange("p (j s) -> p j s", j=c.RPT),
            op=OP.mult,
        )
        srev = sp.tile([c.NT, c.RPT], F32, tag="srev")
        nc.vector.reduce_max(out=srev[:], in_=mk16_3, axis=AX.X)
        # seg_base = (NSEG-1 - srev) * SEGW
        sb = sp.tile([c.NT, c.RPT], F32, tag="sb")
        nc.vector.tensor_scalar(
            out=sb[:],
            in0=srev[:],
            scalar1=-float(c.SEGW),
            scalar2=float((c.NSEG - 1) * c.SEGW),
            op0=OP.mult,
            op1=OP.add,
        )
        # window start (absolute, in padded x): W0 = seg_base + rowbase
        w0 = sp.tile([c.NT, c.RPT], F32, tag="w0")
        nc.vector.tensor_tensor(out=w0[:], in0=sb[:], in1=rbt[:], op=OP.add)

        # pack (w0, score, seg_base) -> [NT, RPT, 3] for the row-major relayout
        P4 = sp.tile([c.NT, c.RPT * 3], F32, tag="P4")
        P43 = P4[:].rearrange("p (j e) -> p j e", e=3)
        for col, src in enumerate((w0, scores, sb)):
            nc.vector.tensor_copy(out=P43[:, :, col : col + 1], in_=src[:, :, None])

        # relayout to rows-on-partitions: RA rows 0..GA-1, RB rows GA..R-1
        RA = sp.tile([GA, 3], F32, tag="RA")
        nta = GA // c.RPT  # tiles covered by group A
        nc.sync.dma_start(out=RA[:], in_=P43[0:nta])
        if c.R > c.P:
            RB = sp.tile([16, 3], F32, tag="RB")
            nc.vector.memset(RB[:], 0.0)
            nc.sync.dma_start(out=RB[0 : c.R - c.P], in_=P43[nta : c.NT])

        # ---- phase 2b/c per row group ---------------------------------------
        def group(Rt, gp, tagp):
            w0u = sp.tile([gp, 1], U32, tag=f"w0u{tagp}")
            nc.vector.tensor_copy(out=w0u[:], in_=Rt[:, 0:1])
            win = sp.tile([gp, c.WINW], F32, tag=f"win{tagp}")
            nc.gpsimd.indirect_dma_start(
                out=win[:],
                out_offset=None,
                in_=xh[:, None],
                in_offset=bass.IndirectOffsetOnAxis(ap=w0u[:, 0:1], axis=0),
            )
            m8 = sp.tile([gp, 8], F32, tag=f"m8{tagp}")
            nc.vector.tensor_copy(out=m8[:], in_=Rt[:, 1:2].to_broadcast([gp, 8]))
            mi = sp.tile([gp, 8], U32, tag=f"mi{tagp}")
            nc.vector.max_index(
                mi[:], m8[:], win[:, c.MARG : c.MARG + c.SEGW]
            )
            ii = sp.tile([gp, 1], F32, tag=f"ii{tagp}")
            nc.vector.tensor_copy(out=ii[:], in_=mi[:, 0:1])

            # neighborhood gather: start = center - W = W0 + ii + 2
            w2 = sp.tile([gp, 1], F32, tag=f"w2{tagp}")
            nc.vector.tensor_tensor(out=w2[:], in0=Rt[:, 0:1], in1=ii[:], op=OP.add)
            nc.vector.tensor_scalar(
                out=w2[:],
                in0=w2[:],
                scalar1=2.0,
                scalar2=float(c.NPAD - c.NBW),
                op0=OP.add,
                op1=OP.min,
            )
            w2u = sp.tile([gp, 1], U32, tag=f"w2u{tagp}")
            nc.vector.tensor_copy(out=w2u[:], in_=w2[:])
            nb = sp.tile([gp, c.NBW], F32, tag=f"nb{tagp}")
            nc.gpsimd.indirect_dma_start(
                out=nb[:],
                out_offset=None,
                in_=xh[:, None],
                in_offset=bass.IndirectOffsetOnAxis(ap=w2u[:, 0:1], axis=0),
            )

            # final math
            O = sp.tile([gp, 3], F32, tag=f"O{tagp}")
            idxm = sp.tile([gp, 1], F32, tag=f"idxm{tagp}")
            nc.vector.tensor_tensor(out=idxm[:], in0=Rt[:, 2:3], in1=ii[:], op=OP.add)
            # py = idx // W via f32 multiply + int cast + +-1 fixup (exact under
            # any f32->int rounding mode); px = idx - py*W.
            t1 = sp.tile([gp, 1], F32, tag=f"t1{tagp}")
            t2 = sp.tile([gp, 1], F32, tag=f"t2{tagp}")
            qi = sp.tile([gp, 1], mybir.dt.int32, tag=f"qi{tagp}")
            nc.vector.tensor_scalar(
                out=t1[:], in0=idxm[:], scalar1=1.0 / c.W, scalar2=0.0013,
                op0=OP.mult, op1=OP.add,
            )
            nc.vector.tensor_copy(out=qi[:], in_=t1[:])
            nc.vector.tensor_copy(out=t1[:], in_=qi[:])  # py candidate (int, f32)
            nc.vector.tensor_scalar(
                out=t2[:], in0=t1[:], scalar1=-float(c.W), scalar2=None, op0=OP.mult
            )
            nc.vector.tensor_tensor(out=t2[:], in0=idxm[:], in1=t2[:], op=OP.add)
            # t2 = idx - cand*W; fix cand by -1 if t2 < 0, +1 if t2 >= W
            lo = sp.tile([gp, 1], F32, tag=f"lo{tagp}")
            nc.vector.tensor_scalar(
                out=lo[:], in0=t2[:], scalar1=0.0, scalar2=None, op0=OP.is_lt
            )
            nc.vector.tensor_tensor(out=t1[:], in0=t1[:], in1=lo[:], op=OP.subtract)
            nc.vector.tensor_scalar(
                out=lo[:], in0=t2[:], scalar1=float(c.W), scalar2=None, op0=OP.is_ge
            )
            nc.vector.tensor_tensor(out=O[:, 1:2], in0=t1[:], in1=lo[:], op=OP.add)
            nc.vector.tensor_scalar(
                out=t2[:], in0=O[:, 1:2], scalar1=-float(c.W), scalar2=None,
                op0=OP.mult,
            )
            nc.vector.tensor_tensor(out=O[:, 0:1], in0=idxm[:], in1=t2[:], op=OP.add)
            mk = sp.tile([gp, 1], F32, tag=f"mk{tagp}")
            nc.vector.tensor_scalar(
                out=mk[:], in0=Rt[:, 1:2], scalar1=0.0, scalar2=None, op0=OP.is_gt
            )
            nc.vector.tensor_tensor(
                out=O[:, 0:2], in0=O[:, 0:2],
                in1=mk[:].to_broadcast([gp, 2]), op=OP.mult,
            )
            # interior = (0 < px < W-1) & (0 < py < H-1)
            hi = sp.tile([gp, 2], F32, tag=f"hi{tagp}")
            nc.vector.memset(hi[:, 0:1], float(c.W - 1))
            nc.vector.memset(hi[:, 1:2], float(c.H - 1))
            ilo = sp.tile([gp, 2], F32, tag=f"ilo{tagp}")
            nc.vector.tensor_scalar(
                out=ilo[:], in0=O[:, 0:2], scalar1=0.0, scalar2=None, op0=OP.is_gt
            )
            ihi = sp.tile([gp, 2], F32, tag=f"ihi{tagp}")
            nc.vector.tensor_tensor(out=ihi[:], in0=O[:, 0:2], in1=hi[:], op=OP.is_lt)
            nc.vector.tensor_tensor(out=ilo[:], in0=ilo[:], in1=ihi[:], op=OP.mult)
            intr = sp.tile([gp, 1], F32, tag=f"intr{tagp}")
            nc.vector.tensor_reduce(out=intr[:], in_=ilo[:], axis=AX.X, op=OP.min)

            # dx = sign(nb[W+1] - nb[W-1]) ; dy = sign(nb[2W] - nb[0])
            D = sp.tile([gp, 2], F32, tag=f"D{tagp}")
            DL = sp.tile([gp, 2], F32, tag=f"DL{tagp}")
            for a, (ir, il) in enumerate(((c.W + 1, c.W - 1), (2 * c.W, 0))):
                nc.vector.tensor_tensor(
                    out=D[:, a : a + 1], in0=nb[:, ir : ir + 1],
                    in1=nb[:, il : il + 1], op=OP.is_gt,
                )
                nc.vector.tensor_tensor(
                    out=DL[:, a : a + 1], in0=nb[:, ir : ir + 1],
                    in1=nb[:, il : il + 1], op=OP.is_lt,
                )
            nc.vector.tensor_tensor(out=D[:], in0=D[:], in1=DL[:], op=OP.subtract)
            nc.vector.tensor_scalar(
                out=D[:], in0=D[:], scalar1=0.25, scalar2=None, op0=OP.mult
            )
            nc.vector.tensor_tensor(
                out=D[:], in0=D[:], in1=intr[:].to_broadcast([gp, 2]), op=OP.mult
            )
            nc.vector.tensor_tensor(out=O[:, 0:2], in0=O[:, 0:2], in1=D[:], op=OP.add)
            nc.vector.tensor_copy(out=O[:, 2:3], in_=Rt[:, 1:2])
            return O

        OA = group(RA[:], GA, "a")
        nc.sync.dma_start(out=oh[0:GA], in_=OA[:])
        if c.R > c.P:
            OB = group(RB[:], 16, "b")
            nc.sync.dma_start(out=oh[c.P : c.R], in_=OB[0 : c.R - c.P])

    nc.compile()
    return nc


def host_constants(cfg: Cfg):
    c = cfg
    r = np.arange(c.R, dtype=np.float64)
    rowbase = (c.FRONT + r * c.HWm - c.MARG).astype(np.float32).reshape(c.NT, c.RPT)
    s = np.arange(c.NSEG, dtype=np.float64)
    row = np.tile((c.NSEG - 1 - s), c.RPT).astype(np.float32)  # [P]
    iotarev = np.tile(row, (c.NT, 1)).astype(np.float32)
    return rowbase, iotarev


def shard_inputs(cfg: Cfg, x: np.ndarray):
    c = cfg
    rowbase, iotarev = host_constants(c)
    in_maps = []
    for k in range(c.ncores):
        shard = np.ascontiguousarray(
            x[k * c.BP : (k + 1) * c.BP], dtype=np.float32
        ).reshape(-1)
        xp = np.zeros(c.NPAD, np.float32)
        xp[c.FRONT : c.FRONT + c.SHN] = shard
        in_maps.append({"x": xp, "rowbase": rowbase, "iotarev": iotarev})
    return in_maps


def assemble_out(cfg: Cfg, per_core_outs):
    c = cfg
    outs = [o.reshape(c.BP, c.C, 3).astype(np.float32) for o in per_core_outs]
    return np.concatenate(outs, axis=0)


_PROGRAM = None


def _program():
    global _PROGRAM
    if _PROGRAM is None:
        _PROGRAM = build_program(CFG)
    return _PROGRAM


def kernel(x: np.ndarray) -> np.ndarray:
    from concourse.bass_utils import run_bass_kernel_spmd

    c = CFG
    assert x.shape == (c.B, c.C, c.H, c.W), x.shape
    nc = _program()
    in_maps = shard_inputs(c, np.asarray(x))
    res = run_bass_kernel_spmd(nc, in_maps, core_ids=list(range(c.ncores)))
    return assemble_out(c, [res.results[k]["out"] for k in range(c.ncores)])



# revision 4
# speedup vs baseline: 1.0842x; 1.0842x over previous
"""Trainium2 Bass kernel for HeatmapMaxDetBlock (argmax + local refinement).

Computes, for x[B, C, H, W]:
    scores = max over (H*W); idx = argmax; px = idx % W, py = idx // W (masked
    by score > 0); quarter-pixel refinement by sign of neighbor differences.
Returns [B, C, 3] = (px, py, scores).

Strategy (pure data parallel over 8 NeuronCores, batch-sharded):
  phase 1: stream the whole shard through SBUF once (17 DMAs of 1.57 MB,
           alternating the two HWDGE rings, 6-deep buffering); one reduce_max
           per tile (alternating DVE / GpSimd) gives per-(row, segment)
           maxima. Partition = segment (NSEG=128), column = row, so the
           transposed maxima land with rows on partitions directly.
  phase 2: split in two chunks (rows 0-63 / 64-135) so the first chunk's
           select + gather + refine work fully overlaps the remaining
           streaming. Per chunk: PE-transpose, row max + winning segment,
           one indirect window gather, max_index for the exact position,
           one neighborhood gather, then the refinement math.
"""

import sys
from contextlib import ExitStack
from dataclasses import dataclass

import numpy as np

for _p in ("/opt/trn_rl_repo",):
    if _p not in sys.path:
        sys.path.insert(0, _p)

import concourse.bass as bass  # noqa: E402
import concourse.tile as tile  # noqa: E402
from concourse import bacc, mybir  # noqa: E402
from concourse.masks import make_identity  # noqa: E402

F32 = mybir.dt.float32
U32 = mybir.dt.uint32
AX = mybir.AxisListType
OP = mybir.AluOpType


@dataclass(frozen=True)
class Cfg:
    B: int = 64
    C: int = 17
    H: int = 256
    W: int = 192
    ncores: int = 8
    P: int = 128
    NSEG: int = 128
    RPD: int = 8  # heatmap rows per DMA
    FRONT: int = 256
    REAR: int = 512

    @property
    def BP(self):  # batches per core
        return self.B // self.ncores

    @property
    def R(self):  # heatmap rows per core
        return self.BP * self.C

    @property
    def HWm(self):
        return self.H * self.W

    @property
    def SEGW(self):
        return self.HWm // self.NSEG

    @property
    def NDMA(self):
        return self.R // self.RPD

    @property
    def MARG(self):
        return self.W + 2

    @property
    def WINW(self):
        return self.SEGW + 2 * self.MARG

    @property
    def NBW(self):
        return 2 * self.W + 1

    @property
    def SHN(self):
        return self.R * self.HWm

    @property
    def NPAD(self):
        return self.FRONT + self.SHN + self.REAR

    @property
    def GA(self):  # rows in chunk A (DMAs 0..NDA-1)
        return self.NDA * self.RPD

    @property
    def NDA(self):  # DMAs in chunk A
        return self.NDMA // 2

    @property
    def GB(self):  # rows in chunk B
        return self.R - self.GA


CFG = Cfg()


def build_program(cfg: Cfg):
    c = cfg
    assert c.NSEG == c.P and c.HWm % c.NSEG == 0 and c.R % c.RPD == 0
    assert c.FRONT >= c.MARG and c.REAR >= c.MARG
    assert 8 <= c.SEGW <= 16384 and c.SEGW % 2 == 0
    assert c.GA <= c.P and c.GB <= c.P

    nc = bacc.Bacc(
        "TRN2", target_bir_lowering=False, debug=False, num_devices=c.ncores
    )
    xh = nc.dram_tensor("x", [c.NPAD], F32, kind="ExternalInput").ap()
    rbh = nc.dram_tensor("rowbase", [c.R, 1], F32, kind="ExternalInput").ap()
    irh = nc.dram_tensor("iotarev", [c.P, c.P], F32, kind="ExternalInput").ap()
    oh = nc.dram_tensor("out", [c.R, 3], F32, kind="ExternalOutput").ap()

    with ExitStack() as ctx:
        tc = ctx.enter_context(tile.TileContext(nc))
        xpool = ctx.enter_context(tc.tile_pool(name="xp", bufs=6))
        sp = ctx.enter_context(tc.tile_pool(name="sp", bufs=1))
        pp = ctx.enter_context(tc.tile_pool(name="pp", bufs=1, space="PSUM"))

        # ---- constants (no deps; scheduler runs them early) ------------------
        ident = sp.tile([c.P, c.P], F32, tag="ident")
        make_identity(nc, ident[:])
        irt = sp.tile([c.P, c.P], F32, tag="irt")
        nc.sync.dma_start(out=irt[:], in_=irh[:])
        rbA = sp.tile([c.GA, 1], F32, tag="rbA")
        nc.sync.dma_start(out=rbA[:], in_=rbh[0 : c.GA])
        rbB = sp.tile([c.GB, 1], F32, tag="rbB")
        nc.sync.dma_start(out=rbB[:], in_=rbh[c.GA : c.R])

        MA = sp.tile([c.P, c.GA], F32, tag="MA")
        MB = sp.tile([c.P, c.GB], F32, tag="MB")

        # ---- phase 1: per-(segment, row) maxima ------------------------------
        # DMA g: [P, RPD*SEGW]; partition p = segment s; free = RPD rows.
        def load_group(g):
            xt = xpool.tile([c.P, c.RPD * c.SEGW], F32, tag="xt")
            off = c.FRONT + g * c.RPD * c.HWm
            src = bass.AP(
                xh.tensor,
                off,
                [[c.SEGW, c.NSEG], [c.HWm, c.RPD], [1, c.SEGW]],
            )
            eng = nc.sync if g % 2 == 0 else nc.scalar
            eng.dma_start(
                out=xt[:].rearrange("p (m u) -> p m u", m=c.RPD), in_=src
            )
            M, col = (MA, g) if g < c.NDA else (MB, g - c.NDA)
            nc.vector.tensor_reduce(
                out=M[:, col * c.RPD : (col + 1) * c.RPD],
                in_=xt[:].rearrange("p (m u) -> p m u", m=c.RPD),
                axis=AX.X,
                op=OP.max,
            )

        # ---- phase 2 for one chunk ------------------------------------------
        def chunk(M, gp, rb, tagp, row0):
            mtp = pp.tile([gp, c.P], F32, tag=f"mtp{tagp}")
            nc.tensor.transpose(out=mtp[:], in_=M[:], identity=ident[:])
            MT = sp.tile([gp, c.P], F32, tag=f"MT{tagp}")
            nc.vector.tensor_copy(out=MT[:], in_=mtp[:])
            # MT[r, s] = max of (row row0+r, segment s)
            scores = sp.tile([gp, 1], F32, tag=f"sc{tagp}")
            nc.vector.tensor_reduce(out=scores[:], in_=MT[:], axis=AX.X, op=OP.max)

            mk = sp.tile([gp, c.P], F32, tag=f"mk{tagp}")
            nc.vector.tensor_tensor(
                out=mk[:],
                in0=MT[:],
                in1=scores[:].to_broadcast([gp, c.P]),
                op=OP.is_equal,
            )
            nc.vector.tensor_tensor(
                out=mk[:], in0=mk[:], in1=irt[0:gp], op=OP.mult
            )
            srev = sp.tile([gp, 1], F32, tag=f"sr{tagp}")
            nc.vector.tensor_reduce(out=srev[:], in_=mk[:], axis=AX.X, op=OP.max)
            # seg_base (in-row) = (NSEG-1 - srev) * SEGW
            sb = sp.tile([gp, 1], F32, tag=f"sb{tagp}")
            nc.vector.tensor_scalar(
                out=sb[:],
                in0=srev[:],
                scalar1=-float(c.SEGW),
                scalar2=float((c.NSEG - 1) * c.SEGW),
                op0=OP.mult,
                op1=OP.add,
            )
            # absolute window start in padded x
            w0 = sp.tile([gp, 1], F32, tag=f"w0{tagp}")
            nc.vector.tensor_tensor(out=w0[:], in0=sb[:], in1=rb[:], op=OP.add)
            w0u = sp.tile([gp, 1], U32, tag=f"w0u{tagp}")
            nc.vector.tensor_copy(out=w0u[:], in_=w0[:])

            win = sp.tile([gp, c.WINW], F32, tag=f"win{tagp}")
            nc.gpsimd.indirect_dma_start(
                out=win[:],
                out_offset=None,
                in_=xh[:, None],
                in_offset=bass.IndirectOffsetOnAxis(ap=w0u[:, 0:1], axis=0),
            )
            m8 = sp.tile([gp, 8], F32, tag=f"m8{tagp}")
            nc.vector.tensor_copy(out=m8[:], in_=scores[:].to_broadcast([gp, 8]))
            mi = sp.tile([gp, 8], U32, tag=f"mi{tagp}")
            nc.vector.max_index(mi[:], m8[:], win[:, c.MARG : c.MARG + c.SEGW])
            ii = sp.tile([gp, 1], F32, tag=f"ii{tagp}")
            nc.vector.tensor_copy(out=ii[:], in_=mi[:, 0:1])

            # neighborhood gather: start = center - W = w0 + ii + 2
            w2 = sp.tile([gp, 1], F32, tag=f"w2{tagp}")
            nc.vector.tensor_tensor(out=w2[:], in0=w0[:], in1=ii[:], op=OP.add)
            nc.vector.tensor_scalar(
                out=w2[:],
                in0=w2[:],
                scalar1=2.0,
                scalar2=float(c.NPAD - c.NBW),
                op0=OP.add,
                op1=OP.min,
            )
            w2u = sp.tile([gp, 1], U32, tag=f"w2u{tagp}")
            nc.vector.tensor_copy(out=w2u[:], in_=w2[:])
            nb = sp.tile([gp, c.NBW], F32, tag=f"nb{tagp}")
            nc.gpsimd.indirect_dma_start(
                out=nb[:],
                out_offset=None,
                in_=xh[:, None],
                in_offset=bass.IndirectOffsetOnAxis(ap=w2u[:, 0:1], axis=0),
            )

            # final math
            O = sp.tile([gp, 3], F32, tag=f"O{tagp}")
            idxm = sp.tile([gp, 1], F32, tag=f"idxm{tagp}")
            nc.vector.tensor_tensor(out=idxm[:], in0=sb[:], in1=ii[:], op=OP.add)
            # py = idx // W via f32 multiply + int cast + +-1 fixup (exact under
            # any f32->int rounding mode); px = idx - py*W.
            t1 = sp.tile([gp, 1], F32, tag=f"t1{tagp}")
            t2 = sp.tile([gp, 1], F32, tag=f"t2{tagp}")
            qi = sp.tile([gp, 1], mybir.dt.int32, tag=f"qi{tagp}")
            nc.vector.tensor_scalar(
                out=t1[:], in0=idxm[:], scalar1=1.0 / c.W, scalar2=0.0013,
                op0=OP.mult, op1=OP.add,
            )
            nc.vector.tensor_copy(out=qi[:], in_=t1[:])
            nc.vector.tensor_copy(out=t1[:], in_=qi[:])  # py candidate (int, f32)
            nc.vector.tensor_scalar(
                out=t2[:], in0=t1[:], scalar1=-float(c.W), scalar2=None, op0=OP.mult
            )
            nc.vector.tensor_tensor(out=t2[:], in0=idxm[:], in1=t2[:], op=OP.add)
            # t2 = idx - cand*W; fix cand by -1 if t2 < 0, +1 if t2 >= W
            lo = sp.tile([gp, 1], F32, tag=f"lo{tagp}")
            nc.vector.tensor_scalar(
                out=lo[:], in0=t2[:], scalar1=0.0, scalar2=None, op0=OP.is_lt
            )
            nc.vector.tensor_tensor(out=t1[:], in0=t1[:], in1=lo[:], op=OP.subtract)
            nc.vector.tensor_scalar(
                out=lo[:], in0=t2[:], scalar1=float(c.W), scalar2=None, op0=OP.is_ge
            )
            nc.vector.tensor_tensor(out=O[:, 1:2], in0=t1[:], in1=lo[:], op=OP.add)
            nc.vector.tensor_scalar(
                out=t2[:], in0=O[:, 1:2], scalar1=-float(c.W), scalar2=None,
                op0=OP.mult,
            )
            nc.vector.tensor_tensor(out=O[:, 0:1], in0=idxm[:], in1=t2[:], op=OP.add)
            mk1 = sp.tile([gp, 1], F32, tag=f"mk1{tagp}")
            nc.vector.tensor_scalar(
                out=mk1[:], in0=scores[:], scalar1=0.0, scalar2=None, op0=OP.is_gt
            )
            nc.vector.tensor_tensor(
                out=O[:, 0:2], in0=O[:, 0:2],
                in1=mk1[:].to_broadcast([gp, 2]), op=OP.mult,
            )
            # interior = (0 < px < W-1) & (0 < py < H-1)
            hi = sp.tile([gp, 2], F32, tag=f"hi{tagp}")
            nc.vector.memset(hi[:, 0:1], float(c.W - 1))
            nc.vector.memset(hi[:, 1:2], float(c.H - 1))
            ilo = sp.tile([gp, 2], F32, tag=f"ilo{tagp}")
            nc.vector.tensor_scalar(
                out=ilo[:], in0=O[:, 0:2], scalar1=0.0, scalar2=None, op0=OP.is_gt
            )
            ihi = sp.tile([gp, 2], F32, tag=f"ihi{tagp}")
            nc.vector.tensor_tensor(out=ihi[:], in0=O[:, 0:2], in1=hi[:], op=OP.is_lt)
            nc.vector.tensor_tensor(out=ilo[:], in0=ilo[:], in1=ihi[:], op=OP.mult)
            intr = sp.tile([gp, 1], F32, tag=f"intr{tagp}")
            nc.vector.tensor_reduce(out=intr[:], in_=ilo[:], axis=AX.X, op=OP.min)

            # dx = sign(nb[W+1] - nb[W-1]) ; dy = sign(nb[2W] - nb[0])
            D = sp.tile([gp, 2], F32, tag=f"D{tagp}")
            DL = sp.tile([gp, 2], F32, tag=f"DL{tagp}")
            for a, (ir, il) in enumerate(((c.W + 1, c.W - 1), (2 * c.W, 0))):
                nc.vector.tensor_tensor(
                    out=D[:, a : a + 1], in0=nb[:, ir : ir + 1],
                    in1=nb[:, il : il + 1], op=OP.is_gt,
                )
                nc.vector.tensor_tensor(
                    out=DL[:, a : a + 1], in0=nb[:, ir : ir + 1],
                    in1=nb[:, il : il + 1], op=OP.is_lt,
                )
            nc.vector.tensor_tensor(out=D[:], in0=D[:], in1=DL[:], op=OP.subtract)
            nc.vector.tensor_scalar(
                out=D[:], in0=D[:], scalar1=0.25, scalar2=None, op0=OP.mult
            )
            nc.vector.tensor_tensor(
                out=D[:], in0=D[:], in1=intr[:].to_broadcast([gp, 2]), op=OP.mult
            )
            nc.vector.tensor_tensor(out=O[:, 0:2], in0=O[:, 0:2], in1=D[:], op=OP.add)
            nc.vector.tensor_copy(out=O[:, 2:3], in_=scores[:])
            nc.sync.dma_start(out=oh[row0 : row0 + gp], in_=O[:])

        for g in range(c.NDA):
            load_group(g)
        chunk(MA, c.GA, rbA, "a", 0)
        for g in range(c.NDA, c.NDMA):
            load_group(g)
        chunk(MB, c.GB, rbB, "b", c.GA)

    nc.compile()
    return nc


def host_constants(cfg: Cfg):
    c = cfg
    r = np.arange(c.R, dtype=np.float64)
    rowbase = (c.FRONT + r * c.HWm - c.MARG).astype(np.float32).reshape(c.R, 1)
    s = np.arange(c.P, dtype=np.float64)
    iotarev = np.tile((c.P - 1 - s).astype(np.float32), (c.P, 1))
    return rowbase, iotarev


def shard_inputs(cfg: Cfg, x: np.ndarray):
    c = cfg
    rowbase, iotarev = host_constants(c)
    in_maps = []
    for k in range(c.ncores):
        shard = np.ascontiguousarray(
            x[k * c.BP : (k + 1) * c.BP], dtype=np.float32
        ).reshape(-1)
        xp = np.zeros(c.NPAD, np.float32)
        xp[c.FRONT : c.FRONT + c.SHN] = shard
        in_maps.append({"x": xp, "rowbase": rowbase, "iotarev": iotarev})
    return in_maps


def assemble_out(cfg: Cfg, per_core_outs):
    c = cfg
    outs = [o.reshape(c.BP, c.C, 3).astype(np.float32) for o in per_core_outs]
    return np.concatenate(outs, axis=0)


_PROGRAM = None


def _program():
    global _PROGRAM
    if _PROGRAM is None:
        _PROGRAM = build_program(CFG)
    return _PROGRAM


def kernel(x: np.ndarray) -> np.ndarray:
    from concourse.bass_utils import run_bass_kernel_spmd

    c = CFG
    assert x.shape == (c.B, c.C, c.H, c.W), x.shape
    nc = _program()
    in_maps = shard_inputs(c, np.asarray(x))
    res = run_bass_kernel_spmd(nc, in_maps, core_ids=list(range(c.ncores)))
    return assemble_out(c, [res.results[k]["out"] for k in range(c.ncores)])
